# revision 41
# baseline (speedup 1.0000x reference)
"""Trainium2 Bass kernel for SAGAN-style self-attention (nn_Attention).

Reference computation (per batch b):
  f = Wf @ x + bf            [32, N]   (N = 64*64 = 4096 pixels)
  g = Wg @ y + bg            [32, N]
  h = Wh @ y + bh            [64, N]
  s[m, n] = sum_c g[c, m] f[c, n]
  beta = softmax(s, axis=n)
  o[m, c] = sum_n beta[m, n] h[c, n]
  out = gamma * o^T + x      [64, N]

Sharding: 8 cores = 4 batches x 2 query-halves. Each core computes the full
softmax rows for its 2048 queries (m) against all 4096 keys (n).

Per-core layout trick: the key/pixel axis is permuted host-side so that the
core's own query half always occupies columns 0:2048 -> the SPMD program is
identical on all cores (no data-dependent slicing).

On-chip algorithm (St orientation: n on partitions, m on free dim):
  St[n, m] = f[:, n].T @ g          (K=32, 4x row-tiled bf16 matmuls)
  E = exp(St)                        (ACT, PSUM->SBUF, bf16 out)
  O'[c|Z, m] = [hT | 1].T @ E        (K=128 accumulated over 32 n-chunks)
  out[c, m] = O'[c, m] * (gamma / Z[m]) + x[c, m]
Softmax max-subtraction is skipped: |s| <= ~8 here, exp is safe in fp32.
Matmul inputs are bf16 (fp32 PSUM accumulation); the residual path and the
softmax normalization stay fp32.
"""
import numpy as np
import ml_dtypes

import bass_rust
import concourse.bass as bass

import concourse.mybir as mybir
import concourse.tile as tile
from concourse.bass_utils import run_bass_kernel_spmd


F32 = mybir.dt.float32
F32R = mybir.dt.float32r
BF16 = mybir.dt.bfloat16
FP8 = mybir.dt.float8e4
FP8E3 = mybir.dt.float8e3
U8 = mybir.dt.uint8
AF = mybir.ActivationFunctionType
DR = mybir.MatmulPerfMode.DoubleRow

# exp(s + EBIAS) keeps E in fp8e4 range: s in [-7.7, 8.6] -> E in [8e-6, 99].
EBIAS = -4.0
# Schraudolph fp8 exp on DVE: u8 = round(A8*s + B8); bitcast e4m3 ~ exp(s-4).
A8 = 8.0 / float(np.log(2.0))
B8 = 56.0 + EBIAS * A8
# E tiles per quad handled by DVE (of 8); rest on ACT. Quad 0 keeps DVE
# free for the projection casts; quads 1/7 run lighter so DVE can absorb
# cast leftovers / the output chains.
def dve_tiles(q):
    if q == 0:
        return ()
    if q in (1, 7):
        return (4, 7)
    return (3, 4, 7)
HSTR = 80  # hT chunk stride (fp8 DoubleRow needs 16B-aligned pair stride)

B, C, N = 4, 64, 4096
M = N // 2              # queries per core
CH = 64
NCH = 32                # number of 128-row key chunks
MCH = 512               # m per matmul (one PSUM bank)


def split_multi_waits(nc, max_waits=1):
    """This walrus build supports a single sync-wait per instruction; spill
    extras onto fresh same-engine NOPs placed right before the instruction."""
    n_spill = 0
    for f in nc.m.functions:
        for bb in f.blocks:
            out = []
            changed = False
            for inst in bb.instructions:
                si = inst.sync_info
                if si is not None and len(si.on_wait) > max_waits:
                    waits = list(si.on_wait)
                    spill, keep = waits[:-max_waits], waits[-max_waits:]
                    for j in range(0, len(spill), max_waits):
                        n_spill += 1
                        out.append(
                            mybir.InstNoOp(
                                name=f"I-waitspill-{n_spill}",
                                engine=inst.engine,
                                bass_nofuse=True,
                                sync_info=mybir.SyncInfo(
                                    on_wait=spill[j : j + max_waits], on_update=[]
                                ),
                            )
                        )
                    inst.sync_info = bass_rust.SyncInfo(
                        on_update=list(si.on_update), on_wait=keep
                    )
                    changed = True
                out.append(inst)
            if changed:
                bb.instructions = out
    return n_spill


def build_kernel():
    nc = bass.Bass("TRN2", target_bir_lowering=False, debug=False, num_devices=8)

    # bf16 inputs are pre-augmented with a ones row (for the bias fold) and
    # pre-permuted so this core's queries are always columns 0:M.
    xab = nc.dram_tensor("xab", [C + 1, N], BF16, kind="ExternalInput").ap()
    yab = nc.dram_tensor("yab", [C + 1, N], BF16, kind="ExternalInput").ap()
    xres = nc.dram_tensor("xres", [C, M], F32, kind="ExternalInput").ap()
    # wcat = wf4 | wg4 | wh  (bf16, one DMA)
    wcat = nc.dram_tensor("wcat", [C + 1, 320], BF16, kind="ExternalInput").ap()
    # wsmall: cols 0:32 ones, col 32 gamma (f32, one DMA)
    wsmall = nc.dram_tensor("wsmall", [128, 33], F32, kind="ExternalInput").ap()
    out = nc.dram_tensor("out", [C, M], F32, kind="ExternalOutput").ap()

    with tile.TileContext(nc) as tc:
        with (
            tc.tile_pool(name="persist", bufs=1) as sb,
            tc.tile_pool(name="epool", bufs=16) as ep,
            tc.tile_pool(name="scratch", bufs=2) as sc,
            tc.tile_pool(name="pst", bufs=2, space="PSUM") as pst,
            tc.tile_pool(name="pacc", bufs=1, space="PSUM") as pacc,
        ):
            # --- tiny dummy exp: trigger the ACT table load ASAP ---
            dm = sc.tile([1, 1], F32, tag="dummy")
            nc.vector.memset(dm[:], 0.0)
            dme = sc.tile([1, 1], F32, tag="dummy")
            nc.scalar.activation(dme[:], dm[:], AF.Exp)

            # --- input DMAs; PE warmup runs off an on-chip memset tile so
            # the clock gate opens with no DMA dependency at all ---
            wwarm_sb = sb.tile([128, 512], BF16)
            nc.vector.memset(wwarm_sb[:], 1.0)
            wcat_sb = sb.tile([C + 1, 320], BF16)
            nc.sync.dma_start(wcat_sb[:], wcat[:])
            wf4_sb = wcat_sb[:, 0:128]
            wg4_sb = wcat_sb[:, 128:256]
            wh_sb = wcat_sb[:, 256:320]
            wsmall_sb = sb.tile([128, 33], F32)
            nc.sync.dma_start(wsmall_sb[:], wsmall[:])
            ones128_sb = wsmall_sb[:, 0:32]
            gam_sb = wsmall_sb[0:1, 32:33]
            gam128 = wsmall_sb[:, 32:33]
            wps = pst.tile([128, 512], F32, tag="st")
            for i in range(9):
                nc.tensor.matmul(
                    wps[:], wwarm_sb[:, 0:128], wwarm_sb[:],
                    start=True, stop=True,
                )
            # x/y split into 1KB-col chunks in consumption order: each
            # dma_start lands on its own HW queue (~39GB/s per queue), so
            # splitting is what buys aggregate bandwidth.
            y_sb = sb.tile([C + 1, N], BF16)
            x_sb = sb.tile([C + 1, N], BF16)
            for j in range(2):
                nc.sync.dma_start(
                    y_sb[:, bass.ts(j, 1024)], yab[:, bass.ts(j, 1024)]
                )
                nc.sync.dma_start(
                    x_sb[:, bass.ts(j, 1024)], xab[:, bass.ts(j, 1024)]
                )
            for j in range(2, 4):
                nc.sync.dma_start(
                    y_sb[:, bass.ts(j, 1024)], yab[:, bass.ts(j, 1024)]
                )
                nc.sync.dma_start(
                    x_sb[:, bass.ts(j, 1024)], xab[:, bass.ts(j, 1024)]
                )
            y_m = y_sb[:, 0:M]
            y_h = y_sb[:, M:N]
            x_m = x_sb[:, 0:M]
            x_h = x_sb[:, M:N]
            xres_sb = sb.tile([C, M], F32)
            nc.sync.dma_start(xres_sb[:], xres[:])

            # --- projections (bf16 matmuls; psum slot shared w/ O' acc) ---
            # g4: [128, M] = 4 stacked copies of g over the core's queries.
            g4_sb = sb.tile([128, M], BF16)
            for half in range(2):
                psg = pst.tile([128, 1024], F32, tag="st", name=f"psg{half}")
                for jj in range(2):
                    nc.tensor.matmul(
                        psg[:, bass.ts(jj, MCH)], wg4_sb,
                        y_m[:, bass.ds(1024 * half + 512 * jj, MCH)],
                        start=True, stop=True,
                    )
                    nc.vector.tensor_copy(
                        g4_sb[:, bass.ds(1024 * half + 512 * jj, MCH)],
                        psg[:, bass.ts(jj, MCH)],
                    )

            # f4: [128, N] = 4 stacked copies of f over all keys. Quad q of
            # the main loop only needs cols 512q:512q+512, so round j > 0 is
            # emitted inside quad 0 (overlaps the first exps).
            f4_sb = sb.tile([128, N], BF16)

            def emit_f4_round(j):
                src_t = x_m if j == 0 else x_h
                for half in range(2):
                    psf = pst.tile(
                        [128, 1024], F32, tag="st", name=f"psf{j}{half}"
                    )
                    for jj in range(2):
                        nc.tensor.matmul(
                            psf[:, bass.ts(jj, MCH)], wf4_sb,
                            src_t[:, bass.ds(1024 * half + 512 * jj, MCH)],
                            start=True, stop=True,
                        )
                        if j == 0 and half == 0:
                            nc.vector.tensor_copy(
                                f4_sb[:, bass.ts(jj, MCH)],
                                psf[:, bass.ts(jj, MCH)],
                            )
                    if j != 0 or half == 1:
                        nc.vector.tensor_copy(
                            f4_sb[:, bass.ds(2048 * j + 1024 * half, 1024)],
                            psf[:],
                        )

            emit_f4_round(0)

            # hT_all (fp8): 32 chunks of [128, HSTR]; cols HSTR*k..+64 = hT of
            # key chunk k (keys on partitions), col HSTR*k+64 = ones (Z col).
            # Chunk stride 80 keeps the DoubleRow pair stride 16B-aligned.
            # Needed only by O', which starts at quad 1 -> emitted in quad 0.
            hT_all = sb.tile([128, 32 * HSTR], FP8)

            def emit_hT_round(t):
                psh = pst.tile([128, 512], F32, tag="st", name=f"psh{t}")
                for u in range(8):
                    k = 8 * t + u
                    ysrc = (
                        y_m[:, bass.ts(k, 128)]
                        if k < 16
                        else y_h[:, bass.ts(k - 16, 128)]
                    )
                    nc.tensor.matmul(
                        psh[:, bass.ds(64 * u, 64)], ysrc, wh_sb,
                        start=True, stop=True,
                    )
                dst = hT_all[:].rearrange("p (k e) -> p k e", k=32)[
                    :, 8 * t : 8 * t + 8, 0:64
                ]
                nc.vector.tensor_scalar_mul(
                    dst, psh[:, 0:512].rearrange("p (a b) -> p a b", a=8),
                    gam128,
                )

            # --- main loop: St -> exp -> O' accumulate ---
            op_ps = None  # allocated after the deferred projections

            opref = {}
            hT_k = hT_all[:].rearrange("p (k e) -> p k e", k=32)

            def emit_op_call(pair, mj, e_t):
                # one fp8 DoubleRow call: contracts key chunks (2*pair,
                # 2*pair+1) = one full E tile, into m bank mj.
                e_ap = e_t[:]
                if e_ap.dtype == U8:
                    e_ap = e_ap.bitcast(FP8)
                nc.tensor.matmul(
                    opref["op"][:, bass.ts(mj, MCH)],
                    hT_k[:, 2 * pair : 2 * pair + 2, 0 : CH + 1],
                    e_ap.rearrange("p (j m) -> p j m", j=2),
                    start=(pair == 0), stop=(pair == 15),
                    perf_mode=DR,
                )

            def emit_oprime_pair(qsrc, h, elist):
                for mj in range(4):
                    emit_op_call(2 * qsrc + h, mj, elist[2 * mj + h])

            # tail chains (declared up front, emitted inside quad 7);
            # one chain per 512-col m bank so each starts the moment its
            # PSUM accumulation stops.
            ones65f = sb.tile([1, CH + 1], F32)
            nc.vector.memset(ones65f[:], 1.0)
            ones65r = sb.tile([1, CH + 1], F32R)
            nc.vector.tensor_scalar_mul(ones65r[:], ones65f[:], 1.0)
            CS = [bass.ds(MCH * i, MCH) for i in range(4)]

            def _t(nm, shape, dt):
                return [
                    sc.tile(shape, dt, tag=f"{nm}{i}", name=f"{nm}{i}")
                    for i in range(4)
                ]

            zrec_all = sc.tile([1, 4 * MCH], F32R, tag="zrecall",
                               name="zrecall")
            zrec = [zrec_all[:, bass.ts(i, MCH)] for i in range(4)]
            rb = _t("rb", [CH, MCH], F32)
            o_sb = _t("osb", [CH, MCH], F32)

            def act_recip(dst, src):
                # ACT-engine reciprocal: ~1.5e-4 max rel err (measured), far
                # inside this kernel's tolerance, and runs on the idle ACT
                # engine in one pass (DVE reciprocal is ~6.5us multi-pass
                # on a single-partition AP).
                eng = nc.scalar
                eng.add_instruction(
                    mybir.InstActivation(
                        name=eng.bass.get_next_instruction_name(),
                        func=AF.Reciprocal,
                        ins=[
                            eng.lower_ap(src),
                            mybir.ImmediateValue(dtype=F32, value=0.0),
                            mybir.ImmediateValue(dtype=F32, value=1.0),
                            mybir.ImmediateValue(dtype=F32, value=0.0),
                        ],
                        outs=[eng.lower_ap(dst)],
                    )
                )

            def emit_chain_recip_all():
                # one 1/Z pass over all four banks: a single ACT instruction
                # = a single Exp->Reciprocal table reload instead of four.
                # (gamma is pre-folded into hT, so rb = 1/Z directly.)
                act_recip(zrec_all[:], opref["op"][CH : CH + 1, 0 : 4 * MCH])

            def emit_chain_rest(i):
                # PE broadcast of 1/Z -> o*rb + xres -> out. Emitted after
                # the final O' call so the rb matmul's wait on ACT's recip
                # can't stall the PE FIFO ahead of remaining O' work.
                op = opref["op"]
                rb_ps = pst.tile([CH + 1, MCH], F32, tag="st", name=f"rbps{i}")
                nc.tensor.matmul(
                    rb_ps[:], ones65r[:], zrec[i],
                    start=True, stop=True,
                )
                nc.vector.tensor_copy(rb[i][:], rb_ps[0:CH, :])
                nc.vector.tensor_mul(
                    o_sb[i][:], op[0:CH, CS[i]], rb[i][:]
                )
                nc.vector.tensor_add(o_sb[i][:], o_sb[i][:], xres_sb[:, CS[i]])
                nc.sync.dma_start(out[:, CS[i]], o_sb[i][:])

            # exp bias const (-4.0) for the ACT path
            ebias_t = sb.tile([128, 1], F32)
            nc.vector.memset(ebias_t[:], EBIAS)

            deferred = [lambda: emit_f4_round(1)] + [
                (lambda t=t: emit_hT_round(t)) for t in range(4)
            ]
            eprev = None
            for q in range(8):
                ecur = []
                for mj in range(4):
                    for h in range(2):
                        # two row-tiled St matmuls (key chunks 4q+2h, 4q+2h+1)
                        st = pst.tile([128, 1024], F32, tag="st")
                        for rr in range(2):
                            r = 2 * h + rr
                            nc.tensor.matmul(
                                st[:, bass.ts(rr, MCH)],
                                f4_sb[
                                    bass.ds(32 * r, 32), bass.ts(4 * q + r, 128)
                                ],
                                g4_sb[bass.ds(32 * r, 32), bass.ts(mj, MCH)],
                                start=True, stop=True,
                                tile_position=(32 * r, 0),
                            )
                        if (2 * mj + h) in dve_tiles(q):
                            # Schraudolph exp: affine to u8, bitcast e4m3
                            e_t = ep.tile([128, 1024], U8, tag="e")
                            nc.vector.tensor_scalar(
                                e_t[:], st[:], A8, B8,
                                mybir.AluOpType.mult, mybir.AluOpType.add,
                            )
                        else:
                            e_t = ep.tile([128, 1024], FP8, tag="e")
                            nc.scalar.activation(
                                e_t[:], st[:], AF.Exp, bias=ebias_t[:]
                            )
                        ecur.append(e_t)
                    if q == 0:
                        # overlap the deferred projections with quad-0 exps
                        for _ in range(2):
                            if deferred:
                                deferred.pop(0)()
                        if mj == 3:
                            onesdst = hT_all[:].rearrange(
                                "p (k e) -> p k e", k=32
                            )[:, :, 64:65]
                            nc.vector.tensor_copy(
                                onesdst,
                                ones128_sb.rearrange(
                                    "p (a b) -> p a b", a=32
                                ),
                            )
                            op_tile = pacc.tile([CH + 1, M], F32, tag="acc")
                            opref["op"] = op_tile
                    elif q < 7:
                        if mj == 0:
                            emit_oprime_pair(q - 1, 0, eprev)
                        elif mj == 2:
                            emit_oprime_pair(q - 1, 1, eprev)
                    else:
                        # quad 7: drain prev pairs up front, then consume own
                        # tiles bank-major so per-bank order ends on pair 15.
                        if mj == 0:
                            emit_oprime_pair(6, 0, eprev)
                            emit_oprime_pair(6, 1, eprev)
                        else:
                            bq = mj - 1  # ecur tiles 2*bq, 2*bq+1 now exist
                            for h in range(2):
                                emit_op_call(14 + h, bq, ecur[2 * bq + h])
                            if mj == 3:
                                # dummy recip: pull the Exp->Reciprocal ACT
                                # table reload off the critical path (runs
                                # while the last O' calls drain)
                                dmr = sc.tile([1, CH + 1], F32, tag="dummy2")
                                act_recip(dmr[:], ones65f[:])
                                for h in range(2):
                                    emit_op_call(14 + h, 3, ecur[6 + h])
                                emit_chain_recip_all()
                                for i in range(4):
                                    emit_chain_rest(i)
                eprev = ecur

    split_multi_waits(nc)
    return nc


def make_in_maps(x, y, Wf, bf, Wg, bg, Wh, bh, gamma):
    x = np.asarray(x, dtype=np.float32).reshape(B, C, N)
    y = np.asarray(y, dtype=np.float32).reshape(B, C, N)
    bf16 = ml_dtypes.bfloat16
    wf4 = np.tile(
        np.concatenate([np.asarray(Wf).T, np.asarray(bf)[None, :]], 0), (1, 4)
    )
    wg4 = np.tile(
        np.concatenate([np.asarray(Wg).T, np.asarray(bg)[None, :]], 0), (1, 4)
    )
    wh = np.concatenate([np.asarray(Wh).T, np.asarray(bh)[None, :]], 0)
    wcat = np.concatenate([wf4, wg4, wh], axis=1).astype(bf16)
    wsmall = np.ones((128, 33), np.float32)
    wsmall[:, 32] = np.float32(np.asarray(gamma).reshape(()))
    onesr = np.ones((1, N), np.float32)

    in_maps = []
    for core in range(8):
        b, half = core // 2, core % 2
        mine = slice(half * M, half * M + M)
        other = slice((1 - half) * M, (1 - half) * M + M)
        xa = np.concatenate([x[b][:, mine], x[b][:, other]], axis=1)
        ya = np.concatenate([y[b][:, mine], y[b][:, other]], axis=1)
        xab = np.concatenate([xa, onesr], axis=0).astype(bf16)
        yab = np.concatenate([ya, onesr], axis=0).astype(bf16)
        in_maps.append(
            {
                "xab": np.ascontiguousarray(xab),
                "yab": np.ascontiguousarray(yab),
                "xres": np.ascontiguousarray(x[b][:, mine]),
                "wcat": wcat, "wsmall": wsmall,
            }
        )
    return in_maps


def assemble_output(results):
    o = np.empty((B, C, N), np.float32)
    for core in range(8):
        b, half = core // 2, core % 2
        o[b][:, half * M : half * M + M] = results[core]["out"]
    return o.reshape(B, C, 64, 64)


_NC_CACHE = {}


def run(trace=False, **inputs):
    if "nc" not in _NC_CACHE:
        _NC_CACHE["nc"] = build_kernel()
    nc = _NC_CACHE["nc"]
    in_maps = make_in_maps(**inputs)
    res = run_bass_kernel_spmd(nc, in_maps, list(range(8)), trace=trace)
    return assemble_output(res.results), res


def kernel(**inputs):
    out, _ = run(trace=False, **inputs)
    return out



# revision 42
# speedup vs baseline: 1.0005x; 1.0005x over previous
"""Trainium2 Bass kernel for SAGAN-style self-attention (nn_Attention).

Reference computation (per batch b):
  f = Wf @ x + bf            [32, N]   (N = 64*64 = 4096 pixels)
  g = Wg @ y + bg            [32, N]
  h = Wh @ y + bh            [64, N]
  s[m, n] = sum_c g[c, m] f[c, n]
  beta = softmax(s, axis=n)
  o[m, c] = sum_n beta[m, n] h[c, n]
  out = gamma * o^T + x      [64, N]

Sharding: 8 cores = 4 batches x 2 query-halves. Each core computes the full
softmax rows for its 2048 queries (m) against all 4096 keys (n).

Per-core layout trick: the key/pixel axis is permuted host-side so that the
core's own query half always occupies columns 0:2048 -> the SPMD program is
identical on all cores (no data-dependent slicing).

On-chip algorithm (St orientation: n on partitions, m on free dim):
  St[n, m] = f[:, n].T @ g          (K=32, 4x row-tiled bf16 matmuls)
  E = exp(St)                        (ACT, PSUM->SBUF, bf16 out)
  O'[c|Z, m] = [hT | 1].T @ E        (K=128 accumulated over 32 n-chunks)
  out[c, m] = O'[c, m] * (gamma / Z[m]) + x[c, m]
Softmax max-subtraction is skipped: |s| <= ~8 here, exp is safe in fp32.
Matmul inputs are bf16 (fp32 PSUM accumulation); the residual path and the
softmax normalization stay fp32.
"""
import numpy as np
import ml_dtypes

import bass_rust
import concourse.bass as bass

import concourse.mybir as mybir
import concourse.tile as tile
from concourse.bass_utils import run_bass_kernel_spmd


F32 = mybir.dt.float32
F32R = mybir.dt.float32r
BF16 = mybir.dt.bfloat16
FP8 = mybir.dt.float8e4
FP8E3 = mybir.dt.float8e3
U8 = mybir.dt.uint8
AF = mybir.ActivationFunctionType
DR = mybir.MatmulPerfMode.DoubleRow

# exp(s + EBIAS) keeps E in fp8e4 range: s in [-7.7, 8.6] -> E in [8e-6, 99].
EBIAS = -4.0
# Schraudolph fp8 exp on DVE: u8 = round(A8*s + B8); bitcast e4m3 ~ exp(s-4).
A8 = 8.0 / float(np.log(2.0))
B8 = 56.0 + EBIAS * A8
# E tiles per quad handled by DVE (of 8); rest on ACT. Quad 0 keeps DVE
# free for the projection casts; quads 1/7 run lighter so DVE can absorb
# cast leftovers / the output chains.
def dve_tiles(q):
    if q == 0:
        return ()
    if q in (1, 7):
        return (4, 7)
    return (3, 4, 7)
HSTR = 80  # hT chunk stride (fp8 DoubleRow needs 16B-aligned pair stride)

B, C, N = 4, 64, 4096
M = N // 2              # queries per core
CH = 64
NCH = 32                # number of 128-row key chunks
MCH = 512               # m per matmul (one PSUM bank)


def split_multi_waits(nc, max_waits=1):
    """This walrus build supports a single sync-wait per instruction; spill
    extras onto fresh same-engine NOPs placed right before the instruction."""
    n_spill = 0
    for f in nc.m.functions:
        for bb in f.blocks:
            out = []
            changed = False
            for inst in bb.instructions:
                si = inst.sync_info
                if si is not None and len(si.on_wait) > max_waits:
                    waits = list(si.on_wait)
                    spill, keep = waits[:-max_waits], waits[-max_waits:]
                    for j in range(0, len(spill), max_waits):
                        n_spill += 1
                        out.append(
                            mybir.InstNoOp(
                                name=f"I-waitspill-{n_spill}",
                                engine=inst.engine,
                                bass_nofuse=True,
                                sync_info=mybir.SyncInfo(
                                    on_wait=spill[j : j + max_waits], on_update=[]
                                ),
                            )
                        )
                    inst.sync_info = bass_rust.SyncInfo(
                        on_update=list(si.on_update), on_wait=keep
                    )
                    changed = True
                out.append(inst)
            if changed:
                bb.instructions = out
    return n_spill


def build_kernel():
    nc = bass.Bass("TRN2", target_bir_lowering=False, debug=False, num_devices=8)

    # bf16 inputs are pre-augmented with a ones row (for the bias fold) and
    # pre-permuted so this core's queries are always columns 0:M.
    xab = nc.dram_tensor("xab", [C + 1, N], BF16, kind="ExternalInput").ap()
    yab = nc.dram_tensor("yab", [C + 1, N], BF16, kind="ExternalInput").ap()
    xres = nc.dram_tensor("xres", [C, M], F32, kind="ExternalInput").ap()
    # wcat = wf4 | wg4 | wh  (bf16, one DMA)
    wcat = nc.dram_tensor("wcat", [C + 1, 320], BF16, kind="ExternalInput").ap()
    # wsmall: cols 0:32 ones, col 32 gamma (f32, one DMA)
    wsmall = nc.dram_tensor("wsmall", [128, 33], F32, kind="ExternalInput").ap()
    out = nc.dram_tensor("out", [C, M], F32, kind="ExternalOutput").ap()

    with tile.TileContext(nc) as tc:
        with (
            tc.tile_pool(name="persist", bufs=1) as sb,
            tc.tile_pool(name="epool", bufs=16) as ep,
            tc.tile_pool(name="scratch", bufs=2) as sc,
            tc.tile_pool(name="pst", bufs=2, space="PSUM") as pst,
            tc.tile_pool(name="pacc", bufs=1, space="PSUM") as pacc,
        ):
            # --- tiny dummy exp: trigger the ACT table load ASAP ---
            dm = sc.tile([1, 1], F32, tag="dummy")
            nc.vector.memset(dm[:], 0.0)
            dme = sc.tile([1, 1], F32, tag="dummy")
            nc.scalar.activation(dme[:], dm[:], AF.Exp)

            # --- input DMAs; PE warmup runs off an on-chip memset tile so
            # the clock gate opens with no DMA dependency at all ---
            wwarm_sb = sb.tile([128, 512], BF16)
            nc.vector.memset(wwarm_sb[:], 1.0)
            wcat_sb = sb.tile([C + 1, 320], BF16)
            nc.sync.dma_start(wcat_sb[:], wcat[:])
            wf4_sb = wcat_sb[:, 0:128]
            wg4_sb = wcat_sb[:, 128:256]
            wh_sb = wcat_sb[:, 256:320]
            wsmall_sb = sb.tile([128, 33], F32)
            nc.sync.dma_start(wsmall_sb[:], wsmall[:])
            ones128_sb = wsmall_sb[:, 0:32]
            gam_sb = wsmall_sb[0:1, 32:33]
            gam128 = wsmall_sb[:, 32:33]
            wps = pst.tile([128, 512], F32, tag="st")
            for i in range(9):
                nc.tensor.matmul(
                    wps[:], wwarm_sb[:, 0:128], wwarm_sb[:],
                    start=True, stop=True,
                )
            # x/y split into 1KB-col chunks in consumption order: each
            # dma_start lands on its own HW queue (~39GB/s per queue), so
            # splitting is what buys aggregate bandwidth.
            y_sb = sb.tile([C + 1, N], BF16)
            x_sb = sb.tile([C + 1, N], BF16)
            for j in range(2):
                nc.sync.dma_start(
                    y_sb[:, bass.ts(j, 1024)], yab[:, bass.ts(j, 1024)]
                )
                nc.sync.dma_start(
                    x_sb[:, bass.ts(j, 1024)], xab[:, bass.ts(j, 1024)]
                )
            for j in range(2, 4):
                nc.sync.dma_start(
                    y_sb[:, bass.ts(j, 1024)], yab[:, bass.ts(j, 1024)]
                )
                nc.sync.dma_start(
                    x_sb[:, bass.ts(j, 1024)], xab[:, bass.ts(j, 1024)]
                )
            y_m = y_sb[:, 0:M]
            y_h = y_sb[:, M:N]
            x_m = x_sb[:, 0:M]
            x_h = x_sb[:, M:N]
            xres_sb = sb.tile([C, M], F32)
            nc.sync.dma_start(xres_sb[:], xres[:])

            # --- projections (bf16 matmuls; psum slot shared w/ O' acc) ---
            # g4: [128, M] = 4 stacked copies of g over the core's queries.
            g4_sb = sb.tile([128, M], BF16)
            for half in range(2):
                psg = pst.tile([128, 1024], F32, tag="st", name=f"psg{half}")
                for jj in range(2):
                    nc.tensor.matmul(
                        psg[:, bass.ts(jj, MCH)], wg4_sb,
                        y_m[:, bass.ds(1024 * half + 512 * jj, MCH)],
                        start=True, stop=True,
                    )
                    nc.vector.tensor_copy(
                        g4_sb[:, bass.ds(1024 * half + 512 * jj, MCH)],
                        psg[:, bass.ts(jj, MCH)],
                    )

            # f4: [128, N] = 4 stacked copies of f over all keys. Quad q of
            # the main loop only needs cols 512q:512q+512, so round j > 0 is
            # emitted inside quad 0 (overlaps the first exps).
            f4_sb = sb.tile([128, N], BF16)

            def emit_f4_round(j):
                src_t = x_m if j == 0 else x_h
                for half in range(2):
                    psf = pst.tile(
                        [128, 1024], F32, tag="st", name=f"psf{j}{half}"
                    )
                    for jj in range(2):
                        nc.tensor.matmul(
                            psf[:, bass.ts(jj, MCH)], wf4_sb,
                            src_t[:, bass.ds(1024 * half + 512 * jj, MCH)],
                            start=True, stop=True,
                        )
                        if j == 0 and half == 0:
                            nc.vector.tensor_copy(
                                f4_sb[:, bass.ts(jj, MCH)],
                                psf[:, bass.ts(jj, MCH)],
                            )
                    if j != 0 or half == 1:
                        nc.vector.tensor_copy(
                            f4_sb[:, bass.ds(2048 * j + 1024 * half, 1024)],
                            psf[:],
                        )

            emit_f4_round(0)

            # hT_all (fp8): 32 chunks of [128, HSTR]; cols HSTR*k..+64 = hT of
            # key chunk k (keys on partitions), col HSTR*k+64 = ones (Z col).
            # Chunk stride 80 keeps the DoubleRow pair stride 16B-aligned.
            # Needed only by O', which starts at quad 1 -> emitted in quad 0.
            hT_all = sb.tile([128, 32 * HSTR], FP8)

            def emit_hT_round(t):
                psh = pst.tile([128, 512], F32, tag="st", name=f"psh{t}")
                for u in range(8):
                    k = 8 * t + u
                    ysrc = (
                        y_m[:, bass.ts(k, 128)]
                        if k < 16
                        else y_h[:, bass.ts(k - 16, 128)]
                    )
                    nc.tensor.matmul(
                        psh[:, bass.ds(64 * u, 64)], ysrc, wh_sb,
                        start=True, stop=True,
                    )
                dst = hT_all[:].rearrange("p (k e) -> p k e", k=32)[
                    :, 8 * t : 8 * t + 8, 0:64
                ]
                nc.vector.tensor_scalar_mul(
                    dst, psh[:, 0:512].rearrange("p (a b) -> p a b", a=8),
                    gam128,
                )

            # --- main loop: St -> exp -> O' accumulate ---
            op_ps = None  # allocated after the deferred projections

            opref = {}
            hT_k = hT_all[:].rearrange("p (k e) -> p k e", k=32)

            def emit_op_call(pair, mj, e_t):
                # one fp8 DoubleRow call: contracts key chunks (2*pair,
                # 2*pair+1) = one full E tile, into m bank mj.
                e_ap = e_t[:].bitcast(FP8)
                nc.tensor.matmul(
                    opref["op"][:, bass.ts(mj, MCH)],
                    hT_k[:, 2 * pair : 2 * pair + 2, 0 : CH + 1],
                    e_ap.rearrange("p (j m) -> p j m", j=2),
                    start=(pair == 0), stop=(pair == 15),
                    perf_mode=DR,
                )

            def emit_oprime_pair(qsrc, h, elist):
                for mj in range(4):
                    emit_op_call(2 * qsrc + h, mj, elist[2 * mj + h])

            # tail chains (declared up front, emitted inside quad 7);
            # one chain per 512-col m bank so each starts the moment its
            # PSUM accumulation stops.
            ones65f = sb.tile([1, CH + 1], F32)
            nc.vector.memset(ones65f[:], 1.0)
            ones65r = sb.tile([1, CH + 1], F32R)
            nc.vector.tensor_scalar_mul(ones65r[:], ones65f[:], 1.0)
            CS = [bass.ds(MCH * i, MCH) for i in range(4)]

            def _t(nm, shape, dt):
                return [
                    sc.tile(shape, dt, tag=f"{nm}{i}", name=f"{nm}{i}")
                    for i in range(4)
                ]

            zrec_all = sc.tile([1, 4 * MCH], F32R, tag="zrecall",
                               name="zrecall")
            zrec = [zrec_all[:, bass.ts(i, MCH)] for i in range(4)]
            rb = _t("rb", [CH, MCH], F32)
            o_sb = _t("osb", [CH, MCH], F32)

            def act_recip(dst, src):
                # ACT-engine reciprocal: ~1.5e-4 max rel err (measured), far
                # inside this kernel's tolerance, and runs on the idle ACT
                # engine in one pass (DVE reciprocal is ~6.5us multi-pass
                # on a single-partition AP).
                eng = nc.scalar
                eng.add_instruction(
                    mybir.InstActivation(
                        name=eng.bass.get_next_instruction_name(),
                        func=AF.Reciprocal,
                        ins=[
                            eng.lower_ap(src),
                            mybir.ImmediateValue(dtype=F32, value=0.0),
                            mybir.ImmediateValue(dtype=F32, value=1.0),
                            mybir.ImmediateValue(dtype=F32, value=0.0),
                        ],
                        outs=[eng.lower_ap(dst)],
                    )
                )

            def emit_chain_recip_all():
                # one 1/Z pass over all four banks: a single ACT instruction
                # = a single Exp->Reciprocal table reload instead of four.
                # (gamma is pre-folded into hT, so rb = 1/Z directly.)
                act_recip(zrec_all[:], opref["op"][CH : CH + 1, 0 : 4 * MCH])

            def emit_chain_rest(i):
                # PE broadcast of 1/Z -> o*rb + xres -> out. Emitted after
                # the final O' call so the rb matmul's wait on ACT's recip
                # can't stall the PE FIFO ahead of remaining O' work.
                op = opref["op"]
                rb_ps = pst.tile([CH + 1, MCH], F32, tag="st", name=f"rbps{i}")
                nc.tensor.matmul(
                    rb_ps[:], ones65r[:], zrec[i],
                    start=True, stop=True,
                )
                nc.vector.tensor_copy(rb[i][:], rb_ps[0:CH, :])
                nc.vector.tensor_mul(
                    o_sb[i][:], op[0:CH, CS[i]], rb[i][:]
                )
                nc.vector.tensor_add(o_sb[i][:], o_sb[i][:], xres_sb[:, CS[i]])
                nc.sync.dma_start(out[:, CS[i]], o_sb[i][:])

            # exp bias const (-4.0) for the ACT path
            ebias_t = sb.tile([128, 1], F32)
            nc.vector.memset(ebias_t[:], EBIAS)

            # E tiles: 16 preallocated buffers reused round-robin (2 quads in
            # flight); manual reuse avoids 64 pool allocations' exit sems.
            e_tiles = [
                sb.tile([128, 1024], U8, name=f"et{i}") for i in range(16)
            ]

            deferred = [lambda: emit_f4_round(1)] + [
                (lambda t=t: emit_hT_round(t)) for t in range(4)
            ]
            eprev = None
            for q in range(8):
                ecur = []
                for mj in range(4):
                    for h in range(2):
                        # two row-tiled St matmuls (key chunks 4q+2h, 4q+2h+1)
                        st = pst.tile([128, 1024], F32, tag="st")
                        for rr in range(2):
                            r = 2 * h + rr
                            nc.tensor.matmul(
                                st[:, bass.ts(rr, MCH)],
                                f4_sb[
                                    bass.ds(32 * r, 32), bass.ts(4 * q + r, 128)
                                ],
                                g4_sb[bass.ds(32 * r, 32), bass.ts(mj, MCH)],
                                start=True, stop=True,
                                tile_position=(32 * r, 0),
                            )
                        e_t = e_tiles[8 * (q % 2) + 2 * mj + h]
                        if (2 * mj + h) in dve_tiles(q):
                            # Schraudolph exp: affine to u8, bitcast e4m3
                            nc.vector.tensor_scalar(
                                e_t[:], st[:], A8, B8,
                                mybir.AluOpType.mult, mybir.AluOpType.add,
                            )
                        else:
                            nc.scalar.activation(
                                e_t[:].bitcast(FP8), st[:], AF.Exp,
                                bias=ebias_t[:],
                            )
                        ecur.append(e_t)
                    if q == 0:
                        # overlap the deferred projections with quad-0 exps
                        for _ in range(2):
                            if deferred:
                                deferred.pop(0)()
                        if mj == 3:
                            onesdst = hT_all[:].rearrange(
                                "p (k e) -> p k e", k=32
                            )[:, :, 64:65]
                            nc.vector.tensor_copy(
                                onesdst,
                                ones128_sb.rearrange(
                                    "p (a b) -> p a b", a=32
                                ),
                            )
                            op_tile = pacc.tile([CH + 1, M], F32, tag="acc")
                            opref["op"] = op_tile
                    elif q < 7:
                        if mj == 0:
                            emit_oprime_pair(q - 1, 0, eprev)
                        elif mj == 2:
                            emit_oprime_pair(q - 1, 1, eprev)
                    else:
                        # quad 7: drain prev pairs up front, then consume own
                        # tiles bank-major so per-bank order ends on pair 15.
                        if mj == 0:
                            emit_oprime_pair(6, 0, eprev)
                            emit_oprime_pair(6, 1, eprev)
                        else:
                            bq = mj - 1  # ecur tiles 2*bq, 2*bq+1 now exist
                            for h in range(2):
                                emit_op_call(14 + h, bq, ecur[2 * bq + h])
                            if mj == 3:
                                # dummy recip: pull the Exp->Reciprocal ACT
                                # table reload off the critical path (runs
                                # while the last O' calls drain)
                                dmr = sc.tile([1, CH + 1], F32, tag="dummy2")
                                act_recip(dmr[:], ones65f[:])
                                for h in range(2):
                                    emit_op_call(14 + h, 3, ecur[6 + h])
                                emit_chain_recip_all()
                                for i in range(4):
                                    emit_chain_rest(i)
                eprev = ecur

    split_multi_waits(nc)
    return nc


def make_in_maps(x, y, Wf, bf, Wg, bg, Wh, bh, gamma):
    x = np.asarray(x, dtype=np.float32).reshape(B, C, N)
    y = np.asarray(y, dtype=np.float32).reshape(B, C, N)
    bf16 = ml_dtypes.bfloat16
    wf4 = np.tile(
        np.concatenate([np.asarray(Wf).T, np.asarray(bf)[None, :]], 0), (1, 4)
    )
    wg4 = np.tile(
        np.concatenate([np.asarray(Wg).T, np.asarray(bg)[None, :]], 0), (1, 4)
    )
    wh = np.concatenate([np.asarray(Wh).T, np.asarray(bh)[None, :]], 0)
    wcat = np.concatenate([wf4, wg4, wh], axis=1).astype(bf16)
    wsmall = np.ones((128, 33), np.float32)
    wsmall[:, 32] = np.float32(np.asarray(gamma).reshape(()))
    onesr = np.ones((1, N), np.float32)

    in_maps = []
    for core in range(8):
        b, half = core // 2, core % 2
        mine = slice(half * M, half * M + M)
        other = slice((1 - half) * M, (1 - half) * M + M)
        xa = np.concatenate([x[b][:, mine], x[b][:, other]], axis=1)
        ya = np.concatenate([y[b][:, mine], y[b][:, other]], axis=1)
        xab = np.concatenate([xa, onesr], axis=0).astype(bf16)
        yab = np.concatenate([ya, onesr], axis=0).astype(bf16)
        in_maps.append(
            {
                "xab": np.ascontiguousarray(xab),
                "yab": np.ascontiguousarray(yab),
                "xres": np.ascontiguousarray(x[b][:, mine]),
                "wcat": wcat, "wsmall": wsmall,
            }
        )
    return in_maps


def assemble_output(results):
    o = np.empty((B, C, N), np.float32)
    for core in range(8):
        b, half = core // 2, core % 2
        o[b][:, half * M : half * M + M] = results[core]["out"]
    return o.reshape(B, C, 64, 64)


_NC_CACHE = {}


def run(trace=False, **inputs):
    if "nc" not in _NC_CACHE:
        _NC_CACHE["nc"] = build_kernel()
    nc = _NC_CACHE["nc"]
    in_maps = make_in_maps(**inputs)
    res = run_bass_kernel_spmd(nc, in_maps, list(range(8)), trace=trace)
    return assemble_output(res.results), res


def kernel(**inputs):
    out, _ = run(trace=False, **inputs)
    return out

